# revision 21
# baseline (speedup 1.0000x reference)
"""GQA causal prefill attention on 8 TRN2 NeuronCores.

Sharding: head-parallel. Core c computes q heads [4c, 4c+4) against kv head c
(n_rep = 4, so the GQA groups align exactly with the shard; no cross-core
communication).

Per-core algorithm (T=2048 tokens, 4 q heads, head_dim 128):
  - Load k, v; build kT (d,s) tiles via PE transpose. v is augmented with a
    ones column -> v_aug (s, 129) in bf16.
  - Per head h: build qT (d,t) via PE transpose; for each s-tile j compute
    S^T_j = k_j @ q_h^T (s=128 partitions, t>=j*128 free) on PE (bf16,
    f32 PSUM), exp(scale*S^T) on ScalarE straight from PSUM into bf16 SBUF
    (causal diagonal tile masked by an upper-triangular multiply).
  - PV with the e^T blocks as the stationary operand and v_aug streaming:
    out_psum (t=128, 129) accumulates over j; column 128 is the softmax
    denominator. Normalize with a per-partition reciprocal multiply and DMA
    the (t, d) tile to DRAM.
"""

import sys
import functools

import numpy as np

if "/opt/trn_rl_repo" not in sys.path:
    sys.path.insert(0, "/opt/trn_rl_repo")

T = 2048
H_TOTAL = 32
N_CORES = 8
H = H_TOTAL // N_CORES  # 4 q heads per core
D = 128
P = 128
NT = T // P  # 16 token tiles
SCALE = 0.08838834764831845

# column offset of s-tile j's slice inside the per-head packed e^T buffer
_EOFF = [0] * (NT + 1)
for _j in range(NT):
    _EOFF[_j + 1] = _EOFF[_j] + (T - P * _j)
E_COLS = _EOFF[NT]  # 17408


def _n_chunks(n_tiles):
    """Split n_tiles 128-col tiles into matmul chunks of <=4 tiles (<=512 cols)."""
    out = []
    i = 0
    while i < n_tiles:
        c = min(4, n_tiles - i)
        out.append((i, c))
        i += c
    return out


def _build_body(tc, nc, q_d, k_d, v_d, o_d, ctx):
    from collections import deque

    import concourse.mybir as mybir
    from concourse.masks import make_identity, make_upper_triangular

    f32 = mybir.dt.float32
    bf16 = mybir.dt.bfloat16

    const = ctx.enter_context(tc.tile_pool(name="const", bufs=1))
    qbp = ctx.enter_context(tc.tile_pool(name="qbf", bufs=4))
    qtp = ctx.enter_context(tc.tile_pool(name="qT", bufs=4))
    ep = ctx.enter_context(tc.tile_pool(name="eT", bufs=2))
    outp = ctx.enter_context(tc.tile_pool(name="outt", bufs=4))
    recp = ctx.enter_context(tc.tile_pool(name="rec", bufs=4))

    # PSUM: two 3-bank S^T units (ping-pong) + two shared 1-bank slots for
    # PV accumulators and transpose staging = exactly 8 banks.
    st_pool = ctx.enter_context(tc.tile_pool(name="st", bufs=2, space="PSUM"))
    sm_pool = ctx.enter_context(tc.tile_pool(name="smp", bufs=2, space="PSUM"))

    identity = const.tile([P, P], bf16, tag="ident")
    make_identity(nc, identity)
    utri = const.tile([P, P], bf16, tag="utri")
    make_upper_triangular(nc, utri, val=1.0, diag=True)

    q_view = q_d.rearrange("(i p) h d -> p i h d", p=P)
    o_view = o_d.rearrange("(i p) h d -> p i h d", p=P)
    NB = NT // 4  # 4-tile batches

    # Prewarm the ACT function table so the first real exp doesn't pay the
    # ~1.3us table load on the critical path.
    warm_sb = recp.tile([P, 1], f32, tag="rec", name="warm")
    nc.scalar.activation(
        out=warm_sb, in_=identity[:, 0:1],
        func=mybir.ActivationFunctionType.Exp,
    )

    # DMA (all on the SP ring, FIFO): reverse-chunked k and q0 — head 0 runs
    # its j-loop descending, so the LAST chunks are needed first and compute
    # starts after only ~0.5MB has landed. v and q1-3 follow.
    k_view = k_d.rearrange("(j p) d -> p j d", p=P)
    k_sb = const.tile([P, NT, D], f32, tag="ksb")
    q0_sb = qbp.tile([P, NT, D], f32, tag="qstg", name="q0stg")
    for b in range(NB - 1, -1, -1):
        nc.sync.dma_start(k_sb[:, 4 * b:4 * b + 4, :], k_view[:, 4 * b:4 * b + 4, :])
        nc.sync.dma_start(q0_sb[:, 4 * b:4 * b + 4, :], q_view[:, 4 * b:4 * b + 4, 0, :])
    v_sb = const.tile([P, NT, D], f32, tag="vsb")
    v_view = v_d.rearrange("(j p) d -> p j d", p=P)
    for b in range(2):
        nc.sync.dma_start(v_sb[:, 8 * b:8 * b + 8, :], v_view[:, 8 * b:8 * b + 8, :])
    q_stg = [None] * H
    q_stg[0] = q0_sb
    for h in range(1, H):
        stg = qbp.tile([P, NT, D], f32, tag="qstg", name=f"q{h}stg")
        nc.sync.dma_start(stg, q_view[:, :, h, :])
        q_stg[h] = stg

    # HAM pre-warm: cheap dummy transposes keep the PE busy during the DMA
    # wait so the clock gate is at 8/8 when real work starts.
    warm_ps = sm_pool.tile([P, 4 * P], bf16, tag="sm", name="warmps")
    for _ in range(32):
        nc.tensor.transpose(warm_ps[0:64, 0:P], identity[:, 0:64], identity)

    def transpose_batch(dst, src, b):
        """Transpose 4 (128,128) bf16 tiles src[:, 4b+m, :] into dst[:, 4b+m, :]
        through one 1-bank PSUM tile and a single batched copy."""
        tp = sm_pool.tile([P, 4 * P], bf16, tag="sm")
        for m in range(4):
            nc.tensor.transpose(tp[:, m * P:(m + 1) * P], src[:, 4 * b + m, :], identity)
        nc.vector.tensor_copy(out=dst[:, 4 * b:4 * b + 4, :], in_=tp)

    q_bf = [None] * H

    def ensure_qbf(h):
        if q_bf[h] is None:
            qb = qbp.tile([P, NT, D], bf16, tag="qbf", name=f"qbf{h}")
            nc.vector.tensor_copy(out=qb, in_=q_stg[h])
            q_bf[h] = qb
        return q_bf[h]

    # Interleave k/q0 chunk casts and transpose batches in DMA-arrival
    # (reverse-chunk) order.
    k_bf = const.tile([P, NT, D], bf16, tag="kbf")
    kT = const.tile([P, NT, P], bf16, tag="kT")  # [d, j, s]
    qb0 = qbp.tile([P, NT, D], bf16, tag="qbf", name="qbf0")
    q_bf[0] = qb0
    qT = [
        qtp.tile([P, NT, P], bf16, tag="qT", name=f"qT{h}") for h in range(H)
    ]  # [d, i, t]
    for b in range(NB - 1, -1, -1):
        nc.vector.tensor_copy(
            out=k_bf[:, 4 * b:4 * b + 4, :], in_=k_sb[:, 4 * b:4 * b + 4, :])
        transpose_batch(kT, k_bf, b)
        nc.vector.tensor_copy(
            out=qb0[:, 4 * b:4 * b + 4, :], in_=q0_sb[:, 4 * b:4 * b + 4, :])
        transpose_batch(qT[0], qb0, b)

    v_aug = const.tile([P, NT, D + 1], bf16, tag="vaug")
    nc.vector.tensor_copy(out=v_aug[:, :, 0:D], in_=v_sb)
    nc.vector.memset(v_aug[:, :, D:D + 1], 1.0)

    # filler thunks: (head, batch) transposes for heads 1..3
    fillers = deque(
        (h, b) for h in range(1, H) for b in range(NT // 4)
    )

    def emit_filler():
        fh, fb = fillers.popleft()
        transpose_batch(qT[fh], ensure_qbf(fh), fb)

    def emit_fillers_for_head(h):
        while fillers and fillers[0][0] <= h:
            emit_filler()

    def emit_chain(eT, h, i):
        """PV accumulation for t-tile i of head h: out_psum (t,129); col 128 is
        the softmax denominator. Normalize and DMA out."""
        pv = sm_pool.tile([P, P + 1], f32, tag="sm")
        for j in range(i + 1):
            c0 = _EOFF[j] + (i - j) * P
            nc.tensor.matmul(
                pv,
                lhsT=eT[:, c0:c0 + P],
                rhs=v_aug[:, j, :],
                start=(j == 0),
                stop=(j == i),
            )
        rec = recp.tile([P, 1], f32, tag="rec")
        nc.vector.reciprocal(rec, pv[:, D:D + 1])
        ot = outp.tile([P, D], f32, tag="outt")
        nc.vector.tensor_scalar_mul(ot, pv[:, 0:D], rec)
        nc.sync.dma_start(o_view[:, i, h, :], ot)

    ready = deque()  # (eT, head, i) PV chains not yet emitted

    def pop_ready(budget, force=False):
        while ready:
            e2, h2, i2 = ready[0]
            size = i2 + 1
            if not force and size > budget and budget < 16:
                break
            ready.popleft()
            emit_chain(e2, h2, i2)
            budget -= size
            if budget <= 0 and not force:
                break

    ST_TILES = 12  # 1536 cols = 3 PSUM banks per S^T unit

    for h in range(H):
        eT = ep.tile([P, E_COLS], bf16, tag="eT")
        # Head 0 walks j DESCENDING: with reverse-chunked DMAs the first step
        # needs only the last k/q chunk, so compute starts ~15us earlier.
        j_order = range(NT - 1, -1, -1) if h == 0 else range(NT)
        for j in j_order:
            # PE work for the exp window FIRST: in-order engine streams mean
            # anything emitted after S^T(j)'s psum-wait would be stuck
            # behind it.
            pop_ready((NT - j) + (16 if h == H - 1 else 3))
            if h == 0 and j < 8 and fillers:
                emit_filler()
            ntiles = NT - j
            off = _EOFF[j]
            if ntiles > ST_TILES:
                g0 = (ntiles + 1) // 2
                groups = [(0, g0), (g0, ntiles - g0)]
            else:
                groups = [(0, ntiles)]
            for (gb, gn) in groups:
                stu = st_pool.tile([P, ST_TILES * P], f32, tag="st")
                for (i0, ci) in _n_chunks(gn):
                    nc.tensor.matmul(
                        stu[:, i0 * P:(i0 + ci) * P],
                        lhsT=kT[:, j, :],
                        rhs=qT[h][:, j + gb + i0:j + gb + i0 + ci, :],
                        start=True,
                        stop=True,
                    )
                nc.scalar.activation(
                    out=eT[:, off + gb * P:off + (gb + gn) * P],
                    in_=stu[:, 0:gn * P],
                    func=mybir.ActivationFunctionType.Exp,
                    scale=SCALE,
                )
            # causal mask on the diagonal tile: keep t_local >= s_local
            nc.vector.tensor_tensor(
                eT[:, off:off + P],
                eT[:, off:off + P],
                utri,
                mybir.AluOpType.mult,
            )
            if h > 0:
                ready.append((eT, h, j))
        if h == 0:
            for i in range(NT):
                ready.append((eT, 0, i))
        if h + 1 < H:
            emit_fillers_for_head(h + 1)
        if h >= 1:
            # everything from head h-1 must drain before its eT slot recycles
            while ready and ready[0][1] < h:
                e2, h2, i2 = ready.popleft()
                emit_chain(e2, h2, i2)
    pop_ready(0, force=True)


@functools.lru_cache(maxsize=1)
def _build():
    import concourse.tile as tile
    import concourse.mybir as mybir
    from concourse import bacc
    from contextlib import ExitStack

    f32 = mybir.dt.float32
    nc = bacc.Bacc(
        "TRN2",
        target_bir_lowering=False,
        debug=False,
        num_devices=N_CORES,
    )
    q_d = nc.dram_tensor("q", (T, H, D), f32, kind="ExternalInput").ap()
    k_d = nc.dram_tensor("k", (T, D), f32, kind="ExternalInput").ap()
    v_d = nc.dram_tensor("v", (T, D), f32, kind="ExternalInput").ap()
    o_d = nc.dram_tensor("out", (T, H, D), f32, kind="ExternalOutput").ap()

    with tile.TileContext(nc) as tc:
        with ExitStack() as ctx:
            _build_body(tc, nc, q_d, k_d, v_d, o_d, ctx)
    nc.compile()
    return nc


def _in_maps(q, k, v):
    q = np.asarray(q, dtype=np.float32)
    k = np.asarray(k, dtype=np.float32)
    v = np.asarray(v, dtype=np.float32)
    return [
        {
            "q": np.ascontiguousarray(q[:, H * c:H * c + H, :]),
            "k": np.ascontiguousarray(k[:, c, :]),
            "v": np.ascontiguousarray(v[:, c, :]),
        }
        for c in range(N_CORES)
    ]


def kernel(q, k, v, _trace=False):
    from concourse.bass_utils import run_bass_kernel_spmd

    nc = _build()
    res = run_bass_kernel_spmd(
        nc, _in_maps(q, k, v), core_ids=list(range(N_CORES)), trace=_trace
    )
    out = np.empty((T, H_TOTAL, D), dtype=np.float32)
    for c in range(N_CORES):
        out[:, H * c:H * c + H, :] = res.results[c]["out"].reshape(T, H, D)
    if _trace:
        return out, res
    return out


# revision 22
# speedup vs baseline: 1.0173x; 1.0173x over previous
"""GQA causal prefill attention on 8 TRN2 NeuronCores.

Sharding: head-parallel. Core c computes q heads [4c, 4c+4) against kv head c
(n_rep = 4, so the GQA groups align exactly with the shard; no cross-core
communication).

Per-core algorithm (T=2048 tokens, 4 q heads, head_dim 128):
  - Load k, v; build kT (d,s) tiles via PE transpose. v is augmented with a
    ones column -> v_aug (s, 129) in bf16.
  - Per head h: build qT (d,t) via PE transpose; for each s-tile j compute
    S^T_j = k_j @ q_h^T (s=128 partitions, t>=j*128 free) on PE (bf16,
    f32 PSUM), exp(scale*S^T) on ScalarE straight from PSUM into bf16 SBUF
    (causal diagonal tile masked by an upper-triangular multiply).
  - PV with the e^T blocks as the stationary operand and v_aug streaming:
    out_psum (t=128, 129) accumulates over j; column 128 is the softmax
    denominator. Normalize with a per-partition reciprocal multiply and DMA
    the (t, d) tile to DRAM.
"""

import sys
import functools

import numpy as np

if "/opt/trn_rl_repo" not in sys.path:
    sys.path.insert(0, "/opt/trn_rl_repo")

T = 2048
H_TOTAL = 32
N_CORES = 8
H = H_TOTAL // N_CORES  # 4 q heads per core
D = 128
P = 128
NT = T // P  # 16 token tiles
SCALE = 0.08838834764831845

# column offset of s-tile j's slice inside the per-head packed e^T buffer
_EOFF = [0] * (NT + 1)
for _j in range(NT):
    _EOFF[_j + 1] = _EOFF[_j] + (T - P * _j)
E_COLS = _EOFF[NT]  # 17408


def _n_chunks(n_tiles):
    """Split n_tiles 128-col tiles into matmul chunks of <=4 tiles (<=512 cols)."""
    out = []
    i = 0
    while i < n_tiles:
        c = min(4, n_tiles - i)
        out.append((i, c))
        i += c
    return out


def _build_body(tc, nc, q_d, k_d, v_d, o_d, ctx):
    from collections import deque

    import concourse.mybir as mybir
    from concourse.masks import make_identity, make_upper_triangular

    f32 = mybir.dt.float32
    bf16 = mybir.dt.bfloat16

    const = ctx.enter_context(tc.tile_pool(name="const", bufs=1))
    qbp = ctx.enter_context(tc.tile_pool(name="qbf", bufs=4))
    qtp = ctx.enter_context(tc.tile_pool(name="qT", bufs=4))
    ep = ctx.enter_context(tc.tile_pool(name="eT", bufs=2))
    outp = ctx.enter_context(tc.tile_pool(name="outt", bufs=4))
    recp = ctx.enter_context(tc.tile_pool(name="rec", bufs=4))

    # PSUM: two 3-bank S^T units (ping-pong) + two shared 1-bank slots for
    # PV accumulators and transpose staging = exactly 8 banks.
    st_pool = ctx.enter_context(tc.tile_pool(name="st", bufs=2, space="PSUM"))
    sm_pool = ctx.enter_context(tc.tile_pool(name="smp", bufs=2, space="PSUM"))

    identity = const.tile([P, P], bf16, tag="ident")
    make_identity(nc, identity)
    utri = const.tile([P, P], bf16, tag="utri")
    make_upper_triangular(nc, utri, val=1.0, diag=True)

    q_view = q_d.rearrange("(i p) h d -> p i h d", p=P)
    o_view = o_d.rearrange("(i p) h d -> p i h d", p=P)
    NB = NT // 4  # 4-tile batches

    # Prewarm the ACT function table so the first real exp doesn't pay the
    # ~1.3us table load on the critical path.
    warm_sb = recp.tile([P, 1], f32, tag="rec", name="warm")
    nc.scalar.activation(
        out=warm_sb, in_=identity[:, 0:1],
        func=mybir.ActivationFunctionType.Exp,
    )

    # DMA (all on the SP ring, FIFO): reverse-chunked k and q0 — head 0 runs
    # its j-loop descending, so the LAST chunks are needed first and compute
    # starts after only ~0.5MB has landed. v and q1-3 follow.
    k_view = k_d.rearrange("(j p) d -> p j d", p=P)
    k_sb = const.tile([P, NT, D], f32, tag="ksb")
    q0_sb = qbp.tile([P, NT, D], f32, tag="qstg", name="q0stg")
    for b in range(NB - 1, -1, -1):
        nc.sync.dma_start(k_sb[:, 4 * b:4 * b + 4, :], k_view[:, 4 * b:4 * b + 4, :])
        nc.sync.dma_start(q0_sb[:, 4 * b:4 * b + 4, :], q_view[:, 4 * b:4 * b + 4, 0, :])
    v_sb = const.tile([P, NT, D], f32, tag="vsb")
    v_view = v_d.rearrange("(j p) d -> p j d", p=P)
    for b in range(2):
        nc.sync.dma_start(v_sb[:, 8 * b:8 * b + 8, :], v_view[:, 8 * b:8 * b + 8, :])
    q_stg = [None] * H
    q_stg[0] = q0_sb
    for h in range(1, H):
        stg = qbp.tile([P, NT, D], f32, tag="qstg", name=f"q{h}stg")
        nc.sync.dma_start(stg, q_view[:, :, h, :])
        q_stg[h] = stg

    # HAM pre-warm: cheap dummy transposes keep the PE busy during the DMA
    # wait so the clock gate is at 8/8 when real work starts.
    warm_ps = sm_pool.tile([P, 4 * P], bf16, tag="sm", name="warmps")
    for _ in range(32):
        nc.tensor.transpose(warm_ps[0:64, 0:P], identity[:, 0:64], identity)

    def transpose_batch(dst, src, b):
        """Transpose 4 (128,128) bf16 tiles src[:, 4b+m, :] into dst[:, 4b+m, :]
        through one 1-bank PSUM tile and a single batched copy."""
        tp = sm_pool.tile([P, 4 * P], bf16, tag="sm")
        for m in range(4):
            nc.tensor.transpose(tp[:, m * P:(m + 1) * P], src[:, 4 * b + m, :], identity)
        nc.vector.tensor_copy(out=dst[:, 4 * b:4 * b + 4, :], in_=tp)

    q_bf = [None] * H

    def ensure_qbf(h):
        if q_bf[h] is None:
            qb = qbp.tile([P, NT, D], bf16, tag="qbf", name=f"qbf{h}")
            nc.vector.tensor_copy(out=qb, in_=q_stg[h])
            q_bf[h] = qb
        return q_bf[h]

    # Interleave k/q0 chunk casts and transpose batches in DMA-arrival
    # (reverse-chunk) order.
    k_bf = const.tile([P, NT, D], bf16, tag="kbf")
    kT = const.tile([P, NT, P], bf16, tag="kT")  # [d, j, s]
    qb0 = qbp.tile([P, NT, D], bf16, tag="qbf", name="qbf0")
    q_bf[0] = qb0
    qT = [
        qtp.tile([P, NT, P], bf16, tag="qT", name=f"qT{h}") for h in range(H)
    ]  # [d, i, t]
    for b in range(NB - 1, -1, -1):
        nc.vector.tensor_copy(
            out=k_bf[:, 4 * b:4 * b + 4, :], in_=k_sb[:, 4 * b:4 * b + 4, :])
        transpose_batch(kT, k_bf, b)
        nc.vector.tensor_copy(
            out=qb0[:, 4 * b:4 * b + 4, :], in_=q0_sb[:, 4 * b:4 * b + 4, :])
        transpose_batch(qT[0], qb0, b)

    v_aug = const.tile([P, NT, D + 1], bf16, tag="vaug")
    nc.vector.tensor_copy(out=v_aug[:, :, 0:D], in_=v_sb)
    nc.vector.memset(v_aug[:, :, D:D + 1], 1.0)

    # filler thunks: (head, batch) transposes for heads 1..3
    fillers = deque(
        (h, b) for h in range(1, H) for b in range(NT // 4)
    )

    def emit_filler():
        fh, fb = fillers.popleft()
        transpose_batch(qT[fh], ensure_qbf(fh), fb)

    def emit_fillers_for_head(h):
        while fillers and fillers[0][0] <= h:
            emit_filler()

    def emit_chain(eT, h, i):
        """PV accumulation for t-tile i of head h: out_psum (t,129); col 128 is
        the softmax denominator. Normalize and DMA out."""
        pv = sm_pool.tile([P, P + 1], f32, tag="sm")
        for j in range(i + 1):
            c0 = _EOFF[j] + (i - j) * P
            nc.tensor.matmul(
                pv,
                lhsT=eT[:, c0:c0 + P],
                rhs=v_aug[:, j, :],
                start=(j == 0),
                stop=(j == i),
            )
        rec = recp.tile([P, 1], f32, tag="rec")
        nc.vector.reciprocal(rec, pv[:, D:D + 1])
        ot = outp.tile([P, D], f32, tag="outt")
        nc.vector.tensor_scalar_mul(ot, pv[:, 0:D], rec)
        nc.sync.dma_start(o_view[:, i, h, :], ot)

    ready = deque()  # (eT, head, i) PV chains not yet emitted

    def pop_ready(budget, force=False):
        while ready:
            e2, h2, i2 = ready[0]
            size = i2 + 1
            if not force and size > budget and budget < 16:
                break
            ready.popleft()
            emit_chain(e2, h2, i2)
            budget -= size
            if budget <= 0 and not force:
                break

    ST_TILES = 12  # 1536 cols = 3 PSUM banks per S^T unit

    for h in range(H):
        eT = ep.tile([P, E_COLS], bf16, tag="eT")
        # Head 0 walks j DESCENDING: with reverse-chunked DMAs the first step
        # needs only the last k/q chunk, so compute starts ~15us earlier.
        j_order = range(NT - 1, -1, -1) if h == 0 else range(NT)
        for j in j_order:
            # PE work for the exp window FIRST: in-order engine streams mean
            # anything emitted after S^T(j)'s psum-wait would be stuck
            # behind it.
            pop_ready((NT - j) + (8 if h == H - 1 else 2))
            if fillers and (
                (h == 0 and j < 8)
                or (0 < h < H - 1 and fillers[0][0] == h + 1 and j % 2 == 0)
            ):
                emit_filler()
            ntiles = NT - j
            off = _EOFF[j]
            if ntiles > ST_TILES:
                g0 = (ntiles + 1) // 2
                groups = [(0, g0), (g0, ntiles - g0)]
            else:
                groups = [(0, ntiles)]
            for (gb, gn) in groups:
                stu = st_pool.tile([P, ST_TILES * P], f32, tag="st")
                for (i0, ci) in _n_chunks(gn):
                    nc.tensor.matmul(
                        stu[:, i0 * P:(i0 + ci) * P],
                        lhsT=kT[:, j, :],
                        rhs=qT[h][:, j + gb + i0:j + gb + i0 + ci, :],
                        start=True,
                        stop=True,
                    )
                nc.scalar.activation(
                    out=eT[:, off + gb * P:off + (gb + gn) * P],
                    in_=stu[:, 0:gn * P],
                    func=mybir.ActivationFunctionType.Exp,
                    scale=SCALE,
                )
            # causal mask on the diagonal tile: keep t_local >= s_local
            nc.vector.tensor_tensor(
                eT[:, off:off + P],
                eT[:, off:off + P],
                utri,
                mybir.AluOpType.mult,
            )
            if h > 0:
                ready.append((eT, h, j))
        if h == 0:
            for i in range(NT):
                ready.append((eT, 0, i))
        if h + 1 < H:
            emit_fillers_for_head(h + 1)
        if h >= 1:
            # everything from head h-1 must drain before its eT slot recycles
            while ready and ready[0][1] < h:
                e2, h2, i2 = ready.popleft()
                emit_chain(e2, h2, i2)
    pop_ready(0, force=True)


@functools.lru_cache(maxsize=1)
def _build():
    import concourse.tile as tile
    import concourse.mybir as mybir
    from concourse import bacc
    from contextlib import ExitStack

    f32 = mybir.dt.float32
    nc = bacc.Bacc(
        "TRN2",
        target_bir_lowering=False,
        debug=False,
        num_devices=N_CORES,
    )
    q_d = nc.dram_tensor("q", (T, H, D), f32, kind="ExternalInput").ap()
    k_d = nc.dram_tensor("k", (T, D), f32, kind="ExternalInput").ap()
    v_d = nc.dram_tensor("v", (T, D), f32, kind="ExternalInput").ap()
    o_d = nc.dram_tensor("out", (T, H, D), f32, kind="ExternalOutput").ap()

    with tile.TileContext(nc) as tc:
        with ExitStack() as ctx:
            _build_body(tc, nc, q_d, k_d, v_d, o_d, ctx)
    nc.compile()
    return nc


def _in_maps(q, k, v):
    q = np.asarray(q, dtype=np.float32)
    k = np.asarray(k, dtype=np.float32)
    v = np.asarray(v, dtype=np.float32)
    return [
        {
            "q": np.ascontiguousarray(q[:, H * c:H * c + H, :]),
            "k": np.ascontiguousarray(k[:, c, :]),
            "v": np.ascontiguousarray(v[:, c, :]),
        }
        for c in range(N_CORES)
    ]


def kernel(q, k, v, _trace=False):
    from concourse.bass_utils import run_bass_kernel_spmd

    nc = _build()
    res = run_bass_kernel_spmd(
        nc, _in_maps(q, k, v), core_ids=list(range(N_CORES)), trace=_trace
    )
    out = np.empty((T, H_TOTAL, D), dtype=np.float32)
    for c in range(N_CORES):
        out[:, H * c:H * c + H, :] = res.results[c]["out"].reshape(T, H, D)
    if _trace:
        return out, res
    return out
